# revision 8
# baseline (speedup 1.0000x reference)
"""Bass/Trainium2 kernel for nn_BDH_GPU_3513283248750 (BDH dense transformer).

Sharding: 8 cores = (B=2) x (H=4). Core i handles batch b=i//4, head h=i%4.
Each layer: per-head compute + AllReduce of the encoder projection across the
4 head-cores of the same batch. All heavy matmuls run as float32r (tf32-like,
~2e-4 relative error) on the PE array.

Self-contained: hardcodes all shapes; only needs the concourse runtime on
PYTHONPATH (provided by the container environment).
"""

import numpy as np

import concourse.bacc as bacc
import concourse.mybir as mybir
import concourse.tile as tile
from concourse.bass_utils import run_bass_kernel_spmd

# model dims
D = 256
H = 4
L = 6
N = 32768
NH = N // H          # 8192 per-head neuron dim
VOCAB = 256
B, T = 2, 2048
EPS = 1e-5

P = 128              # partitions
KB = NH // P         # 64 n-blocks per head
DK = D // P          # 2 d-blocks
CW = 512             # t-chunk width
TC = T // CW         # 4 chunks
TB = T // P          # 16 t-blocks
NCORE = 8

F32 = mybir.dt.float32
F32R = mybir.dt.float32r
F16 = mybir.dt.float16

LAST_RESULT = None   # BassKernelResults of the most recent run (for test.py)
_BUILD_CACHE = {}


def _ln(nc, pool, out_ap, in_ap, eps):
    """LayerNorm rows of [128, D]: out = (x - mean) / sqrt(var + EPS)."""
    stats = pool.tile([P, 6], F32, tag="ln_stats")
    nc.vector.bn_stats(stats, in_ap)
    mv = pool.tile([P, 2], F32, tag="ln_mv")
    nc.vector.bn_aggr(mv, stats)
    rstd = pool.tile([P, 1], F32, tag="ln_rstd")
    nc.scalar.activation(rstd, mv[:, 1:2], mybir.ActivationFunctionType.Sqrt,
                         bias=eps[:, 0:1])
    nc.vector.reciprocal(rstd, rstd)
    nc.vector.tensor_scalar(out_ap, in_ap, mv[:, 0:1], rstd[:, 0:1],
                            mybir.AluOpType.subtract, mybir.AluOpType.mult)


def build(n_layers=L, debug_v=False):
    nc = bacc.Bacc(num_devices=NCORE, dynamic_dma_scratch_size=2048)

    emb_in = nc.dram_tensor("emb", [P, TB, D], F32, kind="ExternalInput")
    wx_in = nc.dram_tensor("wx", [P, DK, NH], F32R, kind="ExternalInput")
    wy_in = nc.dram_tensor("wy", [P, DK, NH], F32R, kind="ExternalInput")
    enc_in = nc.dram_tensor("enc", [P, KB, D], F32R, kind="ExternalInput")
    ro_in = nc.dram_tensor("ro", [P, DK, VOCAB], F32R, kind="ExternalInput")
    cos_in = nc.dram_tensor("costab", [P, KB // 2, T], F16, kind="ExternalInput")
    sin_in = nc.dram_tensor("sintab", [P, KB // 2, T], F16, kind="ExternalInput")
    mask_in = nc.dram_tensor("masks", [P, 4, CW], F16, kind="ExternalInput")
    id_in = nc.dram_tensor("ident", [P, P], F32R, kind="ExternalInput")
    out = nc.dram_tensor("logits", [T, VOCAB], F32, kind="ExternalOutput")
    out_r = out.ap().rearrange("(tb p) v -> p tb v", p=P)
    dbg = None
    if debug_v:
        dbg = nc.dram_tensor("dbg_v", [n_layers + 1, P, TB, D], F32,
                             kind="ExternalOutput")

    with tile.TileContext(nc) as tc:
        with (
            tc.tile_pool(name="const", bufs=1) as cpool,
            tc.tile_pool(name="vres", bufs=1) as vpool,
            tc.tile_pool(name="chunk", bufs=1) as chpool,
            tc.tile_pool(name="s2m", bufs=2) as s2m,
            tc.tile_pool(name="wtab", bufs=2) as wtab,
            tc.tile_pool(name="tabs", bufs=2) as tabs,
            tc.tile_pool(name="stage", bufs=2) as stg,
            tc.tile_pool(name="stage1", bufs=1) as stg1,
            tc.tile_pool(name="small", bufs=4) as sml,
            tc.tile_pool(name="pp256", bufs=4, space="PSUM") as pp256,
            tc.tile_pool(name="pp512", bufs=3, space="PSUM") as pp512,
            tc.tile_pool(name="pptr", bufs=1, space="PSUM") as pptr,
            tc.tile_pool(name="dram", bufs=1, space="DRAM") as dram,
        ):
            ident = cpool.tile([P, P], F32R)
            nc.sync.dma_start(ident, id_in.ap())
            masks = cpool.tile([P, 4, CW], F16)
            nc.sync.dma_start(masks, mask_in.ap())
            eps = cpool.tile([P, 1], F32)
            nc.vector.memset(eps, EPS)

            v_r = vpool.tile([P, TB, D], F32R)      # residual stream (t-part)
            v_f = v_r.bitcast(F32)
            vT = vpool.tile([P, DK, T], F32R)       # transposed (d-part)

            xr_dram = [dram.tile([P, 2, KB // 2, CW], F32R, name=f"xr{i}") for i in range(TC)]
            bnc_in = dram.tile([P, TB, D], F32)
            bnc_out = dram.tile([P, TB, D], F32)

            def v_update(get_block, residual):
                """Set v_r[:, tb, :] (+vT) from LN(chain) per t-block."""
                for tb in range(TB):
                    cur = get_block(tb)
                    if residual:
                        ln1 = stg.tile([P, D], F32, tag="ln1")
                        _ln(nc, sml, ln1, cur, eps)
                        r = stg.tile([P, D], F32, tag="resid")
                        nc.vector.tensor_add(r, v_f[:, tb, :], ln1)
                        _ln(nc, sml, v_r[:, tb, :], r, eps)
                    else:
                        _ln(nc, sml, v_r[:, tb, :], cur, eps)
                    for dk in range(DK):
                        tp = pptr.tile([P, P], F32R, tag="tr")
                        nc.tensor.transpose(
                            tp, v_r[:, tb, dk * P:(dk + 1) * P], ident)
                        nc.vector.tensor_copy(vT[:, dk, tb * P:(tb + 1) * P], tp)

            # ---- initial v = LN(emb) ----
            def emb_block(tb):
                e = stg.tile([P, D], F32, tag="Pt")
                nc.sync.dma_start(e, emb_in.ap()[:, tb, :])
                return e
            v_update(emb_block, residual=False)

            replica_groups = [[0, 1, 2, 3], [4, 5, 6, 7]]

            def dump_v(slot):
                if dbg is not None:
                    nc.sync.dma_start(dbg.ap()[slot], v_f)
            dump_v(0)

            for _layer in range(n_layers):
                for tcx in range(TC):
                    ts = tcx * CW
                    a_ps = [pp256.tile([P, D], F32, tag="a", name=f"a{i}") for i in range(4)]
                    xr_half = chpool.tile([P, KB // 2, CW], F32R)

                    for half in range(2):
                        kb_base = 16 * half
                        # ---- phase 1: xr for this (tc, half) ----
                        for j in range(16):
                            kb1 = kb_base + j          # rope freq index (<32)
                            kb2 = kb1 + 32             # partner block
                            wsl = wtab.tile([P, DK, P], F32R, tag="wx1")
                            nc.sync.dma_start(
                                wsl, wx_in.ap()[:, :, kb1 * P:(kb1 + 1) * P])
                            wsl2 = wtab.tile([P, DK, P], F32R, tag="wx2")
                            nc.sync.dma_start(
                                wsl2, wx_in.ap()[:, :, kb2 * P:(kb2 + 1) * P])
                            ps1 = pp512.tile([P, CW], F32, tag="p512")
                            ps2 = pp512.tile([P, CW], F32, tag="p512")
                            for dk in range(DK):
                                nc.tensor.matmul(
                                    ps1, wsl[:, dk, :], vT[:, dk, ts:ts + CW],
                                    start=(dk == 0), stop=(dk == DK - 1))
                            for dk in range(DK):
                                nc.tensor.matmul(
                                    ps2, wsl2[:, dk, :], vT[:, dk, ts:ts + CW],
                                    start=(dk == 0), stop=(dk == DK - 1))
                            x1 = stg.tile([P, CW], F32, tag="xst1")
                            nc.scalar.activation(
                                x1, ps1, mybir.ActivationFunctionType.Relu)
                            x2 = stg.tile([P, CW], F32, tag="xst2")
                            nc.scalar.activation(
                                x2, ps2, mybir.ActivationFunctionType.Relu)
                            ct = tabs.tile([P, CW], F16, tag="cos")
                            nc.sync.dma_start(ct, cos_in.ap()[:, kb1, ts:ts + CW])
                            st = tabs.tile([P, CW], F16, tag="sin")
                            nc.sync.dma_start(st, sin_in.ap()[:, kb1, ts:ts + CW])
                            m1 = stg.tile([P, CW], F32, tag="m1")
                            nc.vector.tensor_tensor(m1, x2, st, mybir.AluOpType.mult)
                            nc.vector.tensor_tensor(
                                xr_half[:, j, :], x1, ct, mybir.AluOpType.mult)
                            nc.vector.tensor_tensor(
                                xr_half[:, j, :], xr_half[:, j, :], m1,
                                mybir.AluOpType.subtract)
                            m2 = stg.tile([P, CW], F32, tag="m1")
                            nc.vector.tensor_tensor(m2, x1, st, mybir.AluOpType.mult)
                            nc.vector.tensor_tensor(
                                xr_half[:, 16 + j, :], x2, ct, mybir.AluOpType.mult)
                            nc.vector.tensor_tensor(
                                xr_half[:, 16 + j, :], xr_half[:, 16 + j, :], m2,
                                mybir.AluOpType.add)
                            nc.sync.dma_start(
                                xr_dram[tcx][:, half, j, :], xr_half[:, j, :])
                            nc.sync.dma_start(
                                xr_dram[tcx][:, half, 16 + j, :],
                                xr_half[:, 16 + j, :])

                        # ---- phase 2: scoresT blocks + attn accumulation ----
                        for sb in range(4 * tcx + 4):
                            if sb < 4 * tcx:
                                slab = s2m.tile([P, KB // 2, P], F32R, tag="s2m")
                                tcp, off = sb // 4, (sb % 4) * P
                                nc.sync.dma_start(
                                    slab, xr_dram[tcp][:, half, :, off:off + P])
                                lhs = slab
                            else:
                                off = (sb - 4 * tcx) * P
                                lhs = xr_half[:, :, off:off + P]
                            sps = pp512.tile([P, CW], F32, tag="p512")
                            for q in range(KB // 2):
                                nc.tensor.matmul(
                                    sps, lhs[:, q, :], xr_half[:, q, :],
                                    start=(q == 0), stop=(q == KB // 2 - 1))
                            sstage = stg.tile([P, CW], F32R, tag="sst")
                            if sb >= 4 * tcx:
                                nc.vector.tensor_tensor(
                                    sstage, sps, masks[:, sb - 4 * tcx, :],
                                    mybir.AluOpType.mult)
                            else:
                                nc.vector.tensor_copy(sstage, sps)
                            last = (half == 1) and (sb == 4 * tcx + 3)
                            for tr in range(4):
                                nc.tensor.matmul(
                                    a_ps[tr], sstage[:, tr * P:(tr + 1) * P],
                                    v_r[:, sb, :],
                                    start=(half == 0 and sb == 0), stop=last,
                                    skip_group_check=True)

                    # ---- LN(a) -> g; transpose -> gT ----
                    g = stg1.tile([P, 4, D], F32R, tag="g")
                    for tr in range(4):
                        _ln(nc, sml, g[:, tr, :], a_ps[tr], eps)
                    gT = stg1.tile([P, DK, CW], F32R, tag="gT")
                    for tr in range(4):
                        for dk in range(DK):
                            tp = pptr.tile([P, P], F32R, tag="tr")
                            nc.tensor.transpose(
                                tp, g[:, tr, dk * P:(dk + 1) * P], ident)
                            nc.vector.tensor_copy(
                                gT[:, dk, tr * P:(tr + 1) * P], tp)

                    # ---- phase 3: y = relu(g@Wy)*relu(v@Wx); p += y.T@E ----
                    p_ps = [pp256.tile([P, D], F32, tag="a", name=f"p{i}") for i in range(4)]
                    for nb in range(KB):
                        wys = wtab.tile([P, DK, P], F32R, tag="wy1")
                        nc.sync.dma_start(
                            wys, wy_in.ap()[:, :, nb * P:(nb + 1) * P])
                        wxs = wtab.tile([P, DK, P], F32R, tag="wx3")
                        nc.sync.dma_start(
                            wxs, wx_in.ap()[:, :, nb * P:(nb + 1) * P])
                        es = wtab.tile([P, D], F32R, tag="eq")
                        nc.sync.dma_start(es, enc_in.ap()[:, nb, :])
                        psy = pp512.tile([P, CW], F32, tag="p512")
                        for dk in range(DK):
                            nc.tensor.matmul(
                                psy, wys[:, dk, :], gT[:, dk, :],
                                start=(dk == 0), stop=(dk == DK - 1))
                        psx = pp512.tile([P, CW], F32, tag="p512")
                        for dk in range(DK):
                            nc.tensor.matmul(
                                psx, wxs[:, dk, :], vT[:, dk, ts:ts + CW],
                                start=(dk == 0), stop=(dk == DK - 1))
                        ry = stg.tile([P, CW], F32R, tag="ryst")
                        nc.scalar.activation(
                            ry, psy, mybir.ActivationFunctionType.Relu)
                        xs = stg.tile([P, CW], F32, tag="xsst")
                        nc.scalar.activation(
                            xs, psx, mybir.ActivationFunctionType.Relu)
                        nc.vector.tensor_tensor(ry, ry, xs, mybir.AluOpType.mult)
                        for tr in range(4):
                            nc.tensor.matmul(
                                p_ps[tr], ry[:, tr * P:(tr + 1) * P],
                                es,
                                start=(nb == 0), stop=(nb == KB - 1),
                                skip_group_check=True)
                    for tr in range(4):
                        pst = stg.tile([P, D], F32, tag="pst")
                        nc.vector.tensor_copy(pst, p_ps[tr])
                        nc.sync.dma_start(bnc_in[:, 4 * tcx + tr, :], pst)

                # ---- phase 4: AllReduce + v update ----
                nc.gpsimd.collective_compute(
                    "AllReduce", mybir.AluOpType.add,
                    replica_groups=replica_groups,
                    ins=[bnc_in.opt()], outs=[bnc_out.opt()],
                )

                def p_block(tb):
                    pt = stg.tile([P, D], F32, tag="Pt")
                    nc.sync.dma_start(pt, bnc_out[:, tb, :])
                    return pt
                v_update(p_block, residual=True)
                dump_v(_layer + 1)

            # ---- logits = v @ readout ----
            ro = cpool.tile([P, DK, VOCAB], F32R)
            nc.sync.dma_start(ro, ro_in.ap())
            for tb in range(TB):
                lp = pp256.tile([P, VOCAB], F32, tag="a")
                for dk in range(DK):
                    nc.tensor.matmul(
                        lp, vT[:, dk, tb * P:(tb + 1) * P], ro[:, dk, :],
                        start=(dk == 0), stop=(dk == DK - 1))
                ls = stg.tile([P, VOCAB], F32, tag="lst")
                nc.vector.tensor_copy(ls, lp)
                nc.sync.dma_start(out_r[:, tb, :], ls)

    nc.finalize()
    return nc


def _host_inputs(idx, wte, encoder, decoder_x, decoder_y, readout):
    """Per-core input maps implementing the (b, h) sharding."""
    wte = np.asarray(wte, np.float32)
    encoder = np.asarray(encoder, np.float32)
    decoder_x = np.asarray(decoder_x, np.float32)
    decoder_y = np.asarray(decoder_y, np.float32)
    readout = np.asarray(readout, np.float32)
    idx = np.asarray(idx)

    # rope tables, matching the reference's fp32 angle computation
    j = np.arange(NH // 2, dtype=np.float64)
    invf = (1.0 / 10000.0 ** (2.0 * j / NH)).astype(np.float32)
    t = np.arange(T, dtype=np.float32)
    ang = (t[None, :] * invf[:, None]).astype(np.float32)  # [4096, T]
    cosd = np.cos(ang.astype(np.float64))
    sind = np.sin(ang.astype(np.float64))
    # [4096, T] -> [P, 32, T]
    costab = cosd.reshape(KB // 2, P, T).transpose(1, 0, 2).astype(np.float16)
    sintab = sind.reshape(KB // 2, P, T).transpose(1, 0, 2).astype(np.float16)

    # strict-causal masks for diagonal blocks: keep if 128*o + srel < trel
    srel = np.arange(P)[:, None]
    trel = np.arange(CW)[None, :]
    masks = np.stack(
        [(P * o + srel < trel).astype(np.float16) for o in range(4)], axis=1)

    ident = np.eye(P, dtype=np.float32)

    in_maps = []
    for c in range(NCORE):
        b, h = c // 4, c % 4
        emb = wte[idx[b]]                                   # [T, D]
        emb_t = emb.reshape(TB, P, D).transpose(1, 0, 2).copy()
        wx = decoder_x[h]                                   # [D, NH]
        wx_t = wx.reshape(DK, P, NH).transpose(1, 0, 2).copy()
        wy_t = decoder_y[h].reshape(DK, P, NH).transpose(1, 0, 2).copy()
        enc = encoder[h * NH:(h + 1) * NH]                  # [NH, D]
        enc_t = enc.reshape(KB, P, D).transpose(1, 0, 2).copy()
        ro_t = readout.reshape(DK, P, VOCAB).transpose(1, 0, 2).copy()
        in_maps.append({
            "emb": np.ascontiguousarray(emb_t),
            "wx": np.ascontiguousarray(wx_t),
            "wy": np.ascontiguousarray(wy_t),
            "enc": np.ascontiguousarray(enc_t),
            "ro": np.ascontiguousarray(ro_t),
            "costab": np.ascontiguousarray(costab),
            "sintab": np.ascontiguousarray(sintab),
            "masks": np.ascontiguousarray(masks),
            "ident": ident,
        })
    return in_maps


def kernel(idx, wte, encoder, decoder_x, decoder_y, readout,
           _trace=False, _layers=L, _debug_v=False):
    global LAST_RESULT
    key = (_layers, _debug_v)
    if key not in _BUILD_CACHE:
        _BUILD_CACHE[key] = build(_layers, debug_v=_debug_v)
    nc = _BUILD_CACHE[key]
    in_maps = _host_inputs(idx, wte, encoder, decoder_x, decoder_y, readout)
    res = run_bass_kernel_spmd(
        nc, in_maps, core_ids=list(range(NCORE)), trace=_trace)
    LAST_RESULT = res
    out = np.stack([res.results[0]["logits"], res.results[4]["logits"]])
    return out.astype(np.float32)
